# revision 1
# baseline (speedup 1.0000x reference)
"""Trainium2 Bass kernel for CausalSelfAttention (GQA + RoPE + QK-RMSNorm).

Sharding: 8 cores = DP(2 batches) x TP(4 head-groups).
Core c handles batch b=c//4, head group g=c%4 (q heads 4g..4g+3, kv head g).
Device: QKV proj (fp32r), RoPE+RMSNorm, causal attention (max-free softmax —
QK-norm bounds |score| <= sqrt(D)), PV in bf16 with a ones-column appended to
V so row-sums come out of the same matmul, per-512-column AllGather of the
transposed attention output across each 4-core group (overlapped with the
output projection), out-proj for this group's 512 output channels.
Host concatenates the 8 per-core [T, 512] results into [B, T, C].
"""

import sys
import numpy as np

for _p in ("/opt/trn_rl_repo", "/root/.axon_site/_ro/trn_rl_repo"):
    if _p not in sys.path:
        sys.path.append(_p)

import concourse.bass as bass
import concourse.mybir as mybir
import concourse.tile as tile
from concourse import bacc
from concourse.bass_utils import run_bass_kernel_spmd
from concourse.masks import make_identity

F32 = mybir.dt.float32
F32R = mybir.dt.float32r
BF16 = mybir.dt.bfloat16
AF = mybir.ActivationFunctionType
ALU = mybir.AluOpType

B, T, C = 2, 2048, 2048
H, KVH, D = 16, 4, 128
HLOC = H // 4          # q heads per core (TP=4)
DH = HLOC * D          # 512 output channels per core
EPS = 1.1920929e-07
NEG = -1.0e9           # additive causal mask value (exp underflows to 0)
N_CORES = 8

TT = 512               # t-tile (moving free dim)
# dev knobs (not used by the grading path)
CFG = {"skip_ag": False, "phases": 3, "trace_sim": False}
NKC = C // 128         # 16 contraction chunks for the projections
SM_SCALE = float(1.0 / np.sqrt(float(D)))


def r32(ap):
    return ap.bitcast(F32R)


def build_nc(t_seq=T, n_reps=1):
    """Build the SPMD program for one core (all cores run the same program).

    n_reps > 1 replicates the whole body for on-device timing (slope method).
    """
    nc = bacc.Bacc("TRN2", target_bir_lowering=False, debug=False,
                   num_devices=N_CORES)

    xT = nc.dram_tensor("xT", [C, t_seq], F32R, kind="ExternalInput").ap()
    wqT = nc.dram_tensor("wqT", [C, DH], F32R, kind="ExternalInput").ap()
    wkT = nc.dram_tensor("wkT", [C, D], F32R, kind="ExternalInput").ap()
    wvT = nc.dram_tensor("wvT", [C, D], F32R, kind="ExternalInput").ap()
    woT = nc.dram_tensor("woT", [C, DH], F32, kind="ExternalInput").ap()
    cos2 = nc.dram_tensor("cos2", [D, t_seq], F32, kind="ExternalInput").ap()
    sin2s = nc.dram_tensor("sin2s", [D, t_seq], F32, kind="ExternalInput").ap()
    out = nc.dram_tensor("out", [t_seq, DH], F32, kind="ExternalOutput").ap()

    groups = [[0, 1, 2, 3], [4, 5, 6, 7]]

    with tile.TileContext(nc, trace_sim=CFG["trace_sim"]) as tc:
        for _ in range(n_reps):
            build_body(tc, nc, xT, wqT, wkT, wvT, woT, cos2, sin2s, out,
                       groups, t_seq)
    nc.compile()
    return nc


def build_body(tc, nc, xT, wqT, wkT, wvT, woT, cos2, sin2s, out,
               groups, t_seq):
    from contextlib import ExitStack

    tt = TT
    nt = t_seq // tt       # t tiles
    nkb = t_seq // 128     # key blocks

    ctx = ExitStack()
    with ctx:
        # ---------- persistent pools ----------
        const_pool = ctx.enter_context(tc.tile_pool(name="const", bufs=1))
        qk_pool = ctx.enter_context(tc.tile_pool(name="qk", bufs=1))
        yv_pool = ctx.enter_context(tc.tile_pool(name="yv", bufs=1))
        dram = ctx.enter_context(tc.tile_pool(name="dram", bufs=1, space="DRAM"))

        ident = const_pool.tile([128, 128], F32, name="ident")
        make_identity(nc, ident[:])
        # swapmat: [[0, I64], [I64, 0]] — swaps the two D/2 halves via PE
        swapf = const_pool.tile([128, 128], F32, name="swapf")
        nc.gpsimd.memset(swapf[:], 0.0)
        for base in (64, -64):
            nc.gpsimd.affine_select(
                out=swapf[:], in_=swapf[:], compare_op=ALU.not_equal,
                fill=1.0, base=base, pattern=[[-1, 128]], channel_multiplier=1)
        swapmat = const_pool.tile([128, 128], F32R, name="swapmat")
        nc.scalar.activation(swapmat[:], swapf[:], AF.Copy)
        ones_f = const_pool.tile([128, 1], F32, name="ones_f")
        nc.gpsimd.memset(ones_f[:], 1.0)
        ones_col = const_pool.tile([128, 1], F32R, name="ones_col")
        nc.scalar.activation(ones_col[:], ones_f[:], AF.Copy)
        ones_row = const_pool.tile([1, 128], F32R, name="ones_row")
        nc.scalar.activation(ones_row[:], ones_f[0:1, :].to_broadcast([1, 128]),
                             AF.Copy)
        eps_t = const_pool.tile([1, 1], F32, name="eps_t")
        nc.gpsimd.memset(eps_t[:], EPS)
        smsc_f = const_pool.tile([1, 1], F32, name="smsc_f")
        nc.gpsimd.memset(smsc_f[:], SM_SCALE)


        # qT/kT normalized+roped, [D, t_seq] per head
        qTn = [qk_pool.tile([128, t_seq], F32R, name=f"qTn{h}") for h in range(HLOC)]
        kTn = qk_pool.tile([128, t_seq], F32R, name="kTn")
        # per-key-block exp scales: [128, 1] = SM_SCALE / rms(k)[tk]
        rks = [yv_pool.tile([128, 1], F32, name=f"rks{j}") for j in range(nkb)]
        # v_aug: per key block, [128 tk, 129] bf16 (col 128 = 1.0)
        v_aug = [yv_pool.tile([128, 129], BF16, name=f"vaug{j}") for j in range(nkb)]
        # attention output transposed: HLOC head-chunks of [128 c, t_seq]
        yT = [yv_pool.tile([128, t_seq], BF16, name=f"yT{h}") for h in range(HLOC)]

        # ================= phase 1: QKV projections =================
        with (
            tc.tile_pool(name="p1x", bufs=1) as p1x,
            tc.tile_pool(name="p1w", bufs=1) as p1w,
            tc.tile_pool(name="p1t", bufs=2) as p1t,
            tc.tile_pool(name="p1ps", bufs=4, space="PSUM") as p1ps,
            tc.tile_pool(name="p1sw", bufs=2, space="PSUM") as p1sw,
            tc.tile_pool(name="p1ss", bufs=1, space="PSUM") as p1ss,
        ):
            # weights (transposed, c-major) stay resident for phase 1.
            # Interleave x-chunk-0 and weight DMAs per c so the first
            # projection matmul can start after ~0.5 MB instead of ~10 MB.
            wq_sb = [p1w.tile([128, DH], F32R, name=f"wq{c}") for c in range(NKC)]
            wk_sb = [p1w.tile([128, D], F32R, name=f"wk{c}") for c in range(NKC)]
            wv_sb = [p1w.tile([128, D], F32R, name=f"wv{c}") for c in range(NKC)]
            cos_sb = p1w.tile([128, t_seq], F32, name="cos_sb")
            sin_sb = p1w.tile([128, t_seq], F32, name="sin_sb")
            nc.gpsimd.dma_start(cos_sb[:], cos2[:])
            nc.gpsimd.dma_start(sin_sb[:], sin2s[:])
            xt0 = []
            for c in range(NKC):
                xc = p1x.tile([128, tt], F32R, name=f"xt{c}", tag="xt",
                              bufs=NKC)
                nc.sync.dma_start(xc[:], xT[128 * c:128 * (c + 1), 0:tt])
                xt0.append(xc)
                nc.sync.dma_start(wq_sb[c][:], wqT[128 * c:128 * (c + 1), :])
            for c in range(NKC):
                nc.sync.dma_start(wk_sb[c][:], wkT[128 * c:128 * (c + 1), :])
                nc.sync.dma_start(wv_sb[c][:], wvT[128 * c:128 * (c + 1), :])
            vT = p1w.tile([128, t_seq], F32, name="vT")

            for i in range(nt):
                ts = slice(i * tt, (i + 1) * tt)
                # x^T chunk [C, tt] as NKC tiles of [128, tt]
                if i == 0:
                    xt = xt0
                else:
                    xt = []
                    for c in range(NKC):
                        xc = p1x.tile([128, tt], F32R, name=f"xt{c}", tag="xt",
                                      bufs=NKC)
                        nc.sync.dma_start(xc[:], xT[128 * c:128 * (c + 1), ts])
                        xt.append(xc)

                for h in range(HLOC):      # q heads: rope + rmsnorm
                    ps = p1ps.tile([128, tt], F32, name="qkv_ps")
                    for c in range(NKC):
                        nc.tensor.matmul(
                            ps[:], wq_sb[c][:, 128 * h:128 * (h + 1)],
                            xt[c][:], start=(c == 0), stop=(c == NKC - 1))
                    rope_norm(nc, p1t, p1sw, p1ss, ps,
                              cos_sb[:, ts], sin_sb[:, ts],
                              qTn[h][:, ts], swapmat, ones_col, ones_row, eps_t)
                # k head: rope, then 1/rms as per-tk exp scale (not applied
                # to kTn itself — folded into the softmax exp)
                ps = p1ps.tile([128, tt], F32, name="qkv_ps")
                for c in range(NKC):
                    nc.tensor.matmul(ps[:], wk_sb[c][:], xt[c][:],
                                     start=(c == 0), stop=(c == NKC - 1))
                rope_only(nc, p1t, p1sw, p1ss, ps,
                          cos_sb[:, ts], sin_sb[:, ts], kTn[:, ts],
                          swapmat, ones_col, eps_t, smsc_f,
                          [rks[j] for j in range(4 * i, min(4 * i + 4, nkb))],
                          ones_row)
                # v head (no rope/norm)
                ps = p1ps.tile([128, tt], F32, name="qkv_ps")
                for c in range(NKC):
                    nc.tensor.matmul(ps[:], wv_sb[c][:], xt[c][:],
                                     start=(c == 0), stop=(c == NKC - 1))
                nc.scalar.activation(vT[:, ts], ps[:], AF.Copy)

                # v_aug for this chunk: transpose to [tk, d], cast bf16
                for j in range(4 * i, min(4 * i + 4, nkb)):
                    tp = p1sw.tile([128, 128], F32, name="v_tp", tag="sw_ps",
                                   bufs=2)
                    nc.tensor.matmul(tp[:], vT[:, 128 * j:128 * (j + 1)],
                                     ident[:], is_transpose=True)
                    nc.gpsimd.memset(v_aug[j][:, 128:129], 1.0)
                    nc.scalar.activation(v_aug[j][:, 0:128], tp[:], AF.Copy)

        # ================= phase 2+3 shared SBUF =================
        with tc.tile_pool(name="p2m", bufs=1) as p2m:
            masks = []
            for r in range(4):
                m = p2m.tile([128, tt], BF16, name=f"mask{r}")
                nc.gpsimd.memset(m[:], 1.0)
                nc.gpsimd.affine_select(
                    out=m[:], in_=m[:], compare_op=ALU.is_ge, fill=0.0,
                    base=-128 * r, pattern=[[1, tt]], channel_multiplier=-1)
                masks.append(m)

            # wo tiles (DMA overlaps with attention)
            wo_sb = [p2m.tile([128, DH], BF16, name=f"wo{c}") for c in range(NKC)]
            for c in range(NKC):
                nc.gpsimd.dma_start(wo_sb[c][:], woT[128 * c:128 * (c + 1), :])

            ag_in = [dram.tile([DH, tt], BF16, name=f"ag_in{i}") for i in range(nt)]
            ag_out = [dram.tile([4 * DH, tt], BF16, name=f"ag_out{i}")
                      for i in range(nt)]

            # ---------- phase 2: causal attention ----------
            with (
                tc.tile_pool(name="p2pt", bufs=6) as p2pt,
                tc.tile_pool(name="p2on", bufs=6) as p2on,
                tc.tile_pool(name="p2st", bufs=2, space="PSUM") as p2st,
                tc.tile_pool(name="p2o", bufs=1, space="PSUM") as p2o,
                tc.tile_pool(name="p2tp", bufs=1, space="PSUM") as p2tp,
                tc.tile_pool(name="p3y", bufs=1) as p3y,
                tc.tile_pool(name="p3t", bufs=6) as p3t,
            ):
                for i in range(nt if CFG["phases"] >= 2 else 0):
                    nj = min(4 * i + 4, nkb)
                    for h in range(HLOC):
                        o_ps = [p2o.tile([128, 129], F32, name=f"o_ps{t}",
                                         tag=f"o{t}") for t in range(4)]
                        for j in range(nj):
                            r = j - 4 * i
                            # diagonal blocks r=1,2: the first 128*r score
                            # columns are fully masked — skip them (keep the
                            # moving dim >= 256 for full-rate fp32r)
                            off = 128 * r if r in (1, 2) else 0
                            npr = tt - off
                            st = p2st.tile([128, tt], F32, name="st_ps")
                            nc.tensor.matmul(
                                st[:, 0:npr], kTn[:, 128 * j:128 * (j + 1)],
                                qTn[h][:, i * tt + off:(i + 1) * tt])
                            pt = p2pt.tile([128, tt], BF16, name="pt")
                            nc.scalar.activation(pt[:, 0:npr], st[:, 0:npr],
                                                 AF.Exp, scale=rks[j][:])
                            if r >= 0:
                                nc.vector.tensor_mul(pt[:, 0:npr], pt[:, 0:npr],
                                                     masks[r][:, off:tt])
                            for t in range(4):
                                if j <= 4 * i + t:
                                    nc.tensor.matmul(
                                        o_ps[t][:],
                                        pt[:, 128 * t - off:128 * (t + 1) - off],
                                        v_aug[j][:], start=(j == 0),
                                        stop=(j == min(4 * i + t, nj - 1)))
                        for t in range(4):
                            rec = p2on.tile([128, 1], F32, name="rec")
                            nc.vector.reciprocal(rec[:], o_ps[t][:, 128:129])
                            o_n = p2on.tile([128, 128], F32, name="o_n")
                            nc.vector.tensor_scalar_mul(o_n[:],
                                                        o_ps[t][:, 0:128], rec[:])
                            tp = p2tp.tile([128, 128], F32, name="o_tp")
                            nc.tensor.matmul(tp[:], o_n[:], ident[:],
                                             is_transpose=True)
                            nc.vector.tensor_copy(
                                yT[h][:, i * tt + 128 * t:i * tt + 128 * (t + 1)],
                                tp[:])
                    # yT chunk i complete for all heads -> stage + AllGather
                    for h in range(HLOC):
                        nc.sync.dma_start(ag_in[i][128 * h:128 * (h + 1), :],
                                          yT[h][:, i * tt:(i + 1) * tt])
                    if not CFG["skip_ag"]:
                        nc.gpsimd.collective_compute(
                            "AllGather", ALU.bypass, replica_groups=groups,
                            ins=[ag_in[i][:]], outs=[ag_out[i][:]])

                # ---------- phase 3: output projection ----------
                for i in range(nt if CFG["phases"] >= 3 else 0):
                    yf = []
                    for c in range(NKC):
                        yc = p3y.tile([128, tt], BF16, name=f"yf{c}", tag="yf",
                                      bufs=2 * NKC)
                        nc.sync.dma_start(yc[:],
                                          ag_out[i][128 * c:128 * (c + 1), :])
                        yf.append(yc)
                    for t in range(4):
                        ps = p2o.tile([128, DH], F32, name="out_ps", tag="cp")
                        for c in range(NKC):
                            nc.tensor.matmul(
                                ps[:], yf[c][:, 128 * t:128 * (t + 1)],
                                wo_sb[c][:],
                                start=(c == 0), stop=(c == NKC - 1))
                        ot = p3t.tile([128, DH], F32, name="ot")
                        nc.vector.tensor_copy(ot[:], ps[:])
                        nc.sync.dma_start(
                            out[i * tt + 128 * t:i * tt + 128 * (t + 1), :],
                            ot[:])


def rope_only(nc, tmp_pool, sw_pool, ss_pool, ps, cos, sin_s, out_ap,
              swapmat, ones_col, eps_t, smsc, rks_blocks, ones_row):
    # (smsc is a plain-f32 [1,1] constant; the [1,128]x[1,1] transpose
    # matmuls run as fp32 — 1-row cost is negligible)
    """RoPE for k; writes roped (unnormalized) k to out_ap and the per-tk
    exp scales SM_SCALE/rms into rks_blocks ([128,1] each, via PE transpose
    of the [1, tt] reciprocal-rms row)."""
    ttl = ps.shape[-1]
    qf = tmp_pool.tile([128, ttl], F32R, name="qf", tag="qf")
    nc.scalar.activation(qf[:], ps[:], AF.Copy)
    sw = sw_pool.tile([128, ttl], F32, name="sw_ps")
    nc.tensor.matmul(sw[:], swapmat[:], qf[:])
    e1 = tmp_pool.tile([128, ttl], F32, name="e1", tag="e1")
    nc.gpsimd.tensor_mul(e1[:], qf[:], cos)
    qr = tmp_pool.tile([128, ttl], F32, name="qr", tag="qr")
    nc.vector.tensor_mul(qr[:], sw[:], sin_s)
    nc.gpsimd.tensor_add(out_ap, e1[:], qr[:])
    sq = tmp_pool.tile([128, ttl], F32R, name="sq", tag="sq")
    nc.scalar.activation(sq[:], out_ap, AF.Square)
    ss = ss_pool.tile([1, ttl], F32, name="ss_ps", tag="ss")
    nc.tensor.matmul(ss[:], ones_col[:], sq[:])
    sd = tmp_pool.tile([1, ttl], F32, name="sd", tag="sd")
    nc.scalar.activation(sd[:], ss[:], AF.Sqrt, scale=1.0 / 128.0,
                         bias=eps_t[:])
    rr = tmp_pool.tile([1, ttl], F32, name="rr", tag="rr")
    nc.vector.reciprocal(rr[:], sd[:])
    for b, rk in enumerate(rks_blocks):
        rkp = ss_pool.tile([128, 1], F32, name="rk_ps", tag="rb")
        nc.tensor.matmul(rkp[:], rr[0:1, 128 * b:128 * (b + 1)], smsc[:])
        nc.vector.tensor_copy(rk[:], rkp[:])


def rope_norm(nc, tmp_pool, sw_pool, ss_pool, ps, cos, sin_s, out_ap,
              swapmat, ones_col, ones_row, eps_t):
    """RoPE + RMS-norm. ps: [128 d, tt] PSUM (pre-rope head), out_ap: SBUF.

    cos is [cos; cos] (rows duplicated), sin_s is [sin; -sin], so
    rope = ps * cos + swap(ps) * sin_s with the half-swap done on PE.
    """
    ttl = ps.shape[-1]
    # f32r copy of the pre-rope head for the PE half-swap
    qf = tmp_pool.tile([128, ttl], F32R, name="qf", tag="qf")
    nc.scalar.activation(qf[:], ps[:], AF.Copy)
    sw = sw_pool.tile([128, ttl], F32, name="sw_ps")
    nc.tensor.matmul(sw[:], swapmat[:], qf[:])
    e1 = tmp_pool.tile([128, ttl], F32, name="e1", tag="e1")
    nc.gpsimd.tensor_mul(e1[:], qf[:], cos)
    qr = tmp_pool.tile([128, ttl], F32, name="qr", tag="qr")
    nc.vector.tensor_mul(qr[:], sw[:], sin_s)
    nc.gpsimd.tensor_add(qr[:], e1[:], qr[:])
    # sum of squares over d via PE (ones^T @ qr^2)
    sq = tmp_pool.tile([128, ttl], F32R, name="sq", tag="sq")
    nc.scalar.activation(sq[:], qr[:], AF.Square)
    ss = ss_pool.tile([1, ttl], F32, name="ss_ps", tag="ss")
    nc.tensor.matmul(ss[:], ones_col[:], sq[:])
    # rms = sqrt(ss/128 + eps); bcast to 128 partitions via PE; 1/rms on DVE
    sd = tmp_pool.tile([1, ttl], F32R, name="sd", tag="sd")
    nc.scalar.activation(sd[:], ss[:], AF.Sqrt, scale=1.0 / 128.0,
                         bias=eps_t[:])
    rb = ss_pool.tile([128, ttl], F32, name="rb_ps", tag="rb")
    nc.tensor.matmul(rb[:], ones_row[:], sd[:])
    rec = tmp_pool.tile([128, ttl], F32, name="rec", tag="rec")
    nc.vector.reciprocal(rec[:], rb[:])
    nc.vector.tensor_mul(out_ap, qr[:], rec[:])


_NC_CACHE = {}


def get_nc(t_seq=T, n_reps=1):
    key = (t_seq, n_reps)
    if key not in _NC_CACHE:
        _NC_CACHE[key] = build_nc(t_seq, n_reps)
    return _NC_CACHE[key]


def make_in_maps(x, cos, sin, Wq, Wk, Wv, Wo, t_seq=T):
    half = D // 2
    cosT = np.ascontiguousarray(cos.reshape(t_seq, half).T.astype(np.float32))
    sinT = np.ascontiguousarray(sin.reshape(t_seq, half).T.astype(np.float32))
    cos2 = np.concatenate([cosT, cosT], axis=0)
    sin2s = np.concatenate([sinT, -sinT], axis=0)
    wqTs, wkTs, wvTs, woTs = [], [], [], []
    for g in range(4):
        wqTs.append(np.ascontiguousarray(Wq[DH * g:DH * (g + 1), :].T))
        wkTs.append(np.ascontiguousarray(Wk[D * g:D * (g + 1), :].T))
        wvTs.append(np.ascontiguousarray(Wv[D * g:D * (g + 1), :].T))
        woTs.append(np.ascontiguousarray(Wo[DH * g:DH * (g + 1), :].T))
    xTs = [np.ascontiguousarray(x[b].T) for b in range(x.shape[0])]
    in_maps = []
    for c in range(N_CORES):
        b, g = c // 4, c % 4
        in_maps.append({
            "xT": xTs[b], "wqT": wqTs[g], "wkT": wkTs[g], "wvT": wvTs[g],
            "woT": woTs[g], "cos2": cos2, "sin2s": sin2s,
        })
    return in_maps


def kernel(x, cos, sin, Wq, Wk, Wv, Wo):
    x = np.asarray(x, dtype=np.float32)
    nc = get_nc(T)
    in_maps = make_in_maps(x, np.asarray(cos), np.asarray(sin),
                           np.asarray(Wq), np.asarray(Wk), np.asarray(Wv),
                           np.asarray(Wo), T)
    res = run_bass_kernel_spmd(nc, in_maps, core_ids=list(range(N_CORES)))
    outa = np.empty((B, T, C), dtype=np.float32)
    for c in range(N_CORES):
        b, g = c // 4, c % 4
        outa[b, :, DH * g:DH * (g + 1)] = res.results[c]["out"]
    return outa



# revision 13
# speedup vs baseline: 1.0201x; 1.0201x over previous
"""Trainium2 Bass kernel for CausalSelfAttention (GQA + RoPE + QK-RMSNorm).

v2: software-pipelined across the three stages, bf16 matmul operands,
host-packed DMA layouts (one big DMA per tile instead of 16 small ones).

Sharding: 8 cores = DP(2 batches) x TP(4 head-groups).
Core c handles batch b=c//4, head group g=c%4 (q heads 4g..4g+3, kv head g).

Per 512-token tile i, in program order (PE executes in order per engine):
  proj(i):  QKV projections (bf16), RoPE + QK-RMSNorm
  attn(i):  causal attention for q-tile i over key blocks 0..4i+3
            (max-free softmax: |score| <= sqrt(D) after QK-norm; k-side
            1/rms and the 1/sqrt(D) scale are folded into the exp scale;
            a ones-column appended to V yields row-sums from the PV matmul)
  AllGather of the tile's transposed attention output across the 4-core
            TP group (the last tile's AG is split in two halves so the
            out-projection can start earlier)
  outproj(i-1): output projection for the previous tile (its AllGather
            completed during attn(i))

Host packs all DRAM operands so every SBUF tile loads with a single
per-partition-contiguous DMA, and unpacks the [nt*128, 4*DH] output.
"""

import sys
import numpy as np

for _p in ("/opt/trn_rl_repo", "/root/.axon_site/_ro/trn_rl_repo"):
    if _p not in sys.path:
        sys.path.append(_p)

import concourse.bass as bass
import concourse.mybir as mybir
import concourse.tile as tile
from concourse import bacc
from concourse.bass_utils import run_bass_kernel_spmd
from concourse.masks import make_identity

F32 = mybir.dt.float32
F32R = mybir.dt.float32r
BF16 = mybir.dt.bfloat16
AF = mybir.ActivationFunctionType
ALU = mybir.AluOpType

B, T, C = 2, 2048, 2048
H, KVH, D = 16, 4, 128
HLOC = H // 4          # q heads per core (TP=4)
DH = HLOC * D          # 512 output channels per core
EPS = 1.1920929e-07
N_CORES = 8

TT = 512               # t-tile
NT = T // TT
NKC = C // 128         # 16 contraction chunks for the projections
SM_SCALE = float(1.0 / np.sqrt(float(D)))
CFG = {"trace_sim": False}


def build_nc(t_seq=T, n_reps=1):
    nc = bacc.Bacc("TRN2", target_bir_lowering=False, debug=False,
                   num_devices=N_CORES)
    nt = t_seq // TT

    xP = nc.dram_tensor("xP", [128, nt * NKC * TT], BF16,
                        kind="ExternalInput").ap()
    wqP = nc.dram_tensor("wqP", [128, NKC * DH], BF16,
                         kind="ExternalInput").ap()
    wkP = nc.dram_tensor("wkP", [128, NKC * D], BF16,
                         kind="ExternalInput").ap()
    wvP = nc.dram_tensor("wvP", [128, NKC * D], BF16,
                         kind="ExternalInput").ap()
    woP = nc.dram_tensor("woP", [128, NKC * DH], BF16,
                         kind="ExternalInput").ap()
    cosP = nc.dram_tensor("cosP", [128, t_seq], BF16,
                          kind="ExternalInput").ap()
    sinP = nc.dram_tensor("sinP", [128, t_seq], BF16,
                          kind="ExternalInput").ap()
    out = nc.dram_tensor("out", [nt * 128, 4 * DH], F32,
                         kind="ExternalOutput").ap()

    groups = [[0, 1, 2, 3], [4, 5, 6, 7]]

    with tile.TileContext(nc, trace_sim=CFG["trace_sim"]) as tc:
        for _ in range(n_reps):
            build_body(tc, nc, xP, wqP, wkP, wvP, woP, cosP, sinP, out,
                       groups, t_seq)
    nc.compile()
    return nc


def build_body(tc, nc, xP, wqP, wkP, wvP, woP, cosP, sinP, out,
               groups, t_seq):
    from contextlib import ExitStack

    tt = TT
    nt = t_seq // tt
    nkb = t_seq // 128
    last = nt - 1

    ctx = ExitStack()
    with ctx:
        cp = ctx.enter_context(tc.tile_pool(name="cp", bufs=1))
        rg = ctx.enter_context(tc.tile_pool(name="rg", bufs=1))
        ps = ctx.enter_context(tc.tile_pool(name="ps", bufs=1, space="PSUM"))
        dram = ctx.enter_context(tc.tile_pool(name="dram", bufs=1,
                                              space="DRAM"))

        # ---------------- constants ----------------
        identf = cp.tile([128, 128], F32, name="identf")
        make_identity(nc, identf[:])
        identR = cp.tile([128, 128], F32R, name="identR")
        nc.scalar.activation(identR[:], identf[:], AF.Copy)
        # swapmat: [[0, I64], [I64, 0]] — swaps the two D/2 halves via PE
        swapf = cp.tile([128, 128], F32, name="swapf")
        nc.gpsimd.memset(swapf[:], 0.0)
        for base in (64, -64):
            nc.gpsimd.affine_select(
                out=swapf[:], in_=swapf[:], compare_op=ALU.not_equal,
                fill=1.0, base=base, pattern=[[-1, 128]], channel_multiplier=1)
        swap_bf = cp.tile([128, 128], BF16, name="swap_bf")
        nc.scalar.activation(swap_bf[:], swapf[:], AF.Copy)
        ones_f = cp.tile([128, 1], F32, name="ones_f")
        nc.gpsimd.memset(ones_f[:], 1.0)
        ones_col = cp.tile([128, 1], BF16, name="ones_col")
        nc.scalar.activation(ones_col[:], ones_f[:], AF.Copy)
        ones_row = cp.tile([1, 128], F32R, name="ones_row")
        nc.scalar.activation(ones_row[:], ones_f[0:1, :].to_broadcast([1, 128]),
                             AF.Copy)
        eps_t = cp.tile([1, 1], F32, name="eps_t")
        nc.gpsimd.memset(eps_t[:], EPS)
        smsc_t = cp.tile([1, 1], F32, name="smsc_t")
        nc.gpsimd.memset(smsc_t[:], SM_SCALE)

        # ---------------- weights + angle tables (Pool/DVE queues) --------
        wq_all = cp.tile([128, NKC * DH], BF16, name="wq_all")
        hw = NKC * DH // 2
        nc.gpsimd.dma_start(wq_all[:, 0:hw], wqP[:, 0:hw])
        nc.gpsimd.dma_start(wq_all[:, hw:2 * hw], wqP[:, hw:2 * hw])
        wk_all = cp.tile([128, NKC * D], BF16, name="wk_all")
        nc.gpsimd.dma_start(wk_all[:], wkP[:])
        wv_all = cp.tile([128, NKC * D], BF16, name="wv_all")
        nc.gpsimd.dma_start(wv_all[:], wvP[:])

        # masks for the within-block causal triangles
        masks = []
        for r in range(4):
            m = cp.tile([128, tt], BF16, name=f"mask{r}")
            nc.gpsimd.memset(m[:], 1.0)
            nc.gpsimd.affine_select(
                out=m[:], in_=m[:], compare_op=ALU.is_ge, fill=0.0,
                base=-128 * r, pattern=[[1, tt]], channel_multiplier=-1)
            masks.append(m)

        wo_all = cp.tile([128, NKC * DH], BF16, name="wo_all")
        nc.gpsimd.dma_start(wo_all[:], woP[:])

        # ---------------- persistent state ----------------
        kTn = cp.tile([128, t_seq], BF16, name="kTn")
        rks = [cp.tile([128, 1], F32, name=f"rks{j}") for j in range(nkb)]
        v_aug = [cp.tile([128, 129], BF16, name=f"vaug{j}") for j in range(nkb)]

        # dram staging for the AllGathers
        ag_in = [dram.tile([128, HLOC * tt], BF16, name=f"ag_in{i}")
                 for i in range(nt - 1)]
        ag_out = [dram.tile([512, HLOC * tt], BF16, name=f"ag_out{i}")
                  for i in range(nt - 1)]
        ag_in3 = [dram.tile([128, HLOC * tt // 2], BF16, name=f"ag_in3{x}")
                  for x in range(2)]
        ag_out3 = [dram.tile([512, HLOC * tt // 2], BF16, name=f"ag_out3{x}")
                   for x in range(2)]

        # xt for tile 0 (two half DMAs: SP + ACT queues)
        hx = NKC * tt // 2
        xt0 = rg.tile([128, NKC * tt], BF16, name="xt", tag="xt", bufs=2)
        nc.sync.dma_start(xt0[:, 0:hx], xP[:, 0:hx])
        nc.scalar.dma_start(xt0[:, hx:2 * hx], xP[:, hx:2 * hx])
        cos_sb = cp.tile([128, t_seq], BF16, name="cos_sb")
        nc.sync.dma_start(cos_sb[:], cosP[:])
        sin_sb = cp.tile([128, t_seq], BF16, name="sin_sb")
        nc.sync.dma_start(sin_sb[:], sinP[:])

        xt = xt0
        yf = {}            # k -> sbuf tile holding all of ag_out[k]
        yf3 = [None, None]
        qTn = [None] * HLOC

        def rope_common(acc, csl, ssl):
            """shared RoPE front: returns (e1+qr writer inputs) qf,e1,qr."""
            qf = rg.tile([128, tt], BF16, name="qf", tag="qf", bufs=2)
            nc.scalar.activation(qf[:], acc[:], AF.Copy)
            swp = ps.tile([128, tt], F32, name="swp", tag="sup", bufs=2)
            nc.tensor.matmul(swp[:], swap_bf[:], qf[:])
            e1 = rg.tile([128, tt], BF16, name="e1", tag="e1", bufs=2)
            nc.gpsimd.tensor_mul(e1[:], qf[:], csl)
            qr = rg.tile([128, tt], BF16, name="qr", tag="qr", bufs=2)
            nc.vector.tensor_mul(qr[:], swp[:], ssl)
            return e1, qr

        def sumsq(src_ap):
            """ones^T @ src^2 -> [1, tt] psum; returns (ss_psum_ap, sq)."""
            sq = rg.tile([128, tt], BF16, name="sq", tag="sq", bufs=2)
            nc.scalar.activation(sq[:], src_ap, AF.Square)
            ssp = ps.tile([128, tt], F32, name="ssp", tag="sup", bufs=2)
            nc.tensor.matmul(ssp[0:1, :], ones_col[:], sq[:])
            return ssp

        for i in range(nt):
            ts = slice(i * tt, (i + 1) * tt)
            csl = cos_sb[:, ts]
            ssl = sin_sb[:, ts]

            # ================= proj(i) =================
            for h in range(HLOC):
                acc = ps.tile([128, tt], F32, name="acc", tag="acc", bufs=2)
                for c in range(NKC):
                    nc.tensor.matmul(
                        acc[:], wq_all[:, DH * c + 128 * h:DH * c + 128 * (h + 1)],
                        xt[:, tt * c:tt * (c + 1)],
                        start=(c == 0), stop=(c == NKC - 1))
                e1, qr = rope_common(acc, csl, ssl)
                qr2 = rg.tile([128, tt], BF16, name="qr2", tag="qr2", bufs=2)
                nc.vector.tensor_add(qr2[:], e1[:], qr[:])
                ssp = sumsq(qr2[:])
                sd = rg.tile([1, tt], F32R, name="sd", tag="sd", bufs=2)
                nc.scalar.activation(sd[:], ssp[0:1, :], AF.Sqrt,
                                     scale=1.0 / 128.0, bias=eps_t[:])
                rbp = ps.tile([128, tt], F32, name="rbp", tag="sup", bufs=2)
                nc.tensor.matmul(rbp[:], ones_row[:], sd[:])
                rec = rg.tile([128, tt], F32, name="rec", tag="rec", bufs=2)
                nc.vector.reciprocal(rec[:], rbp[:])
                qh = rg.tile([128, tt], BF16, name=f"qTn{h}", tag=f"qTn{h}",
                             bufs=2)
                nc.vector.tensor_mul(qh[:], qr2[:], rec[:])
                qTn[h] = qh

            # k head: rope; 1/rms folded into the exp scale (rks)
            acc = ps.tile([128, tt], F32, name="acc", tag="acc", bufs=2)
            for c in range(NKC):
                nc.tensor.matmul(acc[:], wk_all[:, D * c:D * (c + 1)],
                                 xt[:, tt * c:tt * (c + 1)],
                                 start=(c == 0), stop=(c == NKC - 1))
            e1, qr = rope_common(acc, csl, ssl)
            nc.vector.tensor_add(kTn[:, ts], e1[:], qr[:])
            ssp = sumsq(kTn[:, ts])
            sd = rg.tile([1, tt], F32R, name="sd", tag="sd", bufs=2)
            nc.scalar.activation(sd[:], ssp[0:1, :], AF.Sqrt,
                                 scale=1.0 / 128.0, bias=eps_t[:])
            rr = rg.tile([1, tt], F32, name="rr", tag="rr", bufs=2)
            nc.vector.reciprocal(rr[:], sd[:])
            for b in range(4):
                rkp = ps.tile([128, tt], F32, name="rkp", tag="sup", bufs=2)
                nc.tensor.matmul(rkp[:, 0:1], rr[0:1, 128 * b:128 * (b + 1)],
                                 smsc_t[:])
                nc.vector.tensor_copy(rks[4 * i + b][:], rkp[:, 0:1])

            # v head
            acc = ps.tile([128, tt], F32, name="acc", tag="acc", bufs=2)
            for c in range(NKC):
                nc.tensor.matmul(acc[:], wv_all[:, D * c:D * (c + 1)],
                                 xt[:, tt * c:tt * (c + 1)],
                                 start=(c == 0), stop=(c == NKC - 1))
            vTt = rg.tile([128, tt], F32R, name="vTt", tag="vTt", bufs=2)
            nc.scalar.activation(vTt[:], acc[:], AF.Copy)
            for b in range(4):
                j = 4 * i + b
                tpp = ps.tile([128, tt], F32, name="tpp", tag="sup", bufs=2)
                nc.tensor.matmul(tpp[:, 0:128].bitcast(F32R),
                                 vTt[:, 128 * b:128 * (b + 1)], identR[:],
                                 is_transpose=True)
                nc.gpsimd.memset(v_aug[j][:, 128:129], 1.0)
                nc.scalar.activation(v_aug[j][:, 0:128], tpp[:, 0:128],
                                     AF.Copy)

            # ================= attn(i) =================
            # prefetch DMAs for the NEXT pipeline steps (issued up front so
            # their queues fill while PE works):
            if i + 1 < nt:
                xt = rg.tile([128, NKC * tt], BF16, name="xt", tag="xt",
                             bufs=2)
                nc.sync.dma_start(
                    xt[:], xP[:, (i + 1) * NKC * tt:(i + 2) * NKC * tt])
            if i >= 1 and i - 1 < nt - 1:
                # load all of ag_out[i-1] for outproj(i-1) in one DMA (Pool
                # queue; waits on AG(i-1), which lands early in attn(i))
                t_ = rg.tile([128, 4 * HLOC * tt], BF16, name="yfa",
                             tag="yfa", bufs=2)
                nc.gpsimd.dma_start(
                    t_[:], ag_out[i - 1][:].rearrange("(g p) c -> p g c", g=4))
                yf[i - 1] = t_

            yT_all = rg.tile([128, HLOC * tt], BF16, name="yT", tag="yT",
                             bufs=2)
            nj = 4 * i + 4
            for h in range(HLOC):
                # four PV accumulators, each alone in its PSUM bank (two
                # borrow the proj accumulator ring, idle during attention)
                o4 = ([ps.tile([128, tt], F32, name=f"o2{x}", tag=f"o2{x}",
                               bufs=1) for x in range(2)]
                      + [ps.tile([128, tt], F32, name="o2acc", tag="acc",
                                 bufs=2) for _ in range(2)])
                for j in range(nj):
                    r = j - 4 * i
                    off = 128 * r if r >= 0 else 0
                    npr = tt - off
                    st = ps.tile([128, tt], F32, name="st", tag="stx", bufs=2)
                    nc.tensor.matmul(st[:, 0:npr], kTn[:, 128 * j:128 * (j + 1)],
                                     qTn[h][:, off:tt])
                    pt = rg.tile([128, tt], BF16, name="pt", tag="pt", bufs=4)
                    nc.scalar.activation(pt[:, 0:npr], st[:, 0:npr], AF.Exp,
                                         scale=rks[j][:])
                    if r >= 0:
                        nc.vector.tensor_mul(pt[:, 0:npr], pt[:, 0:npr],
                                             masks[r][:, off:tt])
                    for t in range(4):
                        if j <= 4 * i + t:
                            nc.tensor.matmul(
                                o4[t][:, 0:129],
                                pt[:, 128 * t - off:128 * (t + 1) - off],
                                v_aug[j][:], start=(j == 0),
                                stop=(j == 4 * i + t))
                for t in range(4):
                    o2t = o4[t]
                    ro = rg.tile([128, 1], F32, name="ro", tag="ro", bufs=2)
                    nc.vector.reciprocal(ro[:], o2t[:, 128:129])
                    o_n = rg.tile([128, 128], F32R, name="o_n", tag="o_n",
                                  bufs=2)
                    nc.vector.tensor_scalar_mul(o_n[:],
                                                o2t[:, 0:128], ro[:])
                    tpp = ps.tile([128, tt], F32, name="tpp", tag="sup",
                                  bufs=2)
                    nc.tensor.matmul(tpp[:, 0:128].bitcast(F32R),
                                     o_n[:], identR[:],
                                     is_transpose=True)
                    nc.vector.tensor_copy(
                        yT_all[:, tt * h + 128 * t:tt * h + 128 * (t + 1)],
                        tpp[:, 0:128])
                if i == last and h == 1:
                    # first-half AllGather of the last tile (heads 0-1)
                    nc.sync.dma_start(ag_in3[0][:], yT_all[:, 0:2 * tt])
                    nc.gpsimd.collective_compute(
                        "AllGather", ALU.bypass, replica_groups=groups,
                        ins=[ag_in3[0][:]], outs=[ag_out3[0][:]])
                    yf3[0] = rg.tile([128, 2 * HLOC * tt], BF16, name="y3a",
                                     tag="y3a", bufs=1)
                    nc.sync.dma_start(
                        yf3[0][:],
                        ag_out3[0][:].rearrange("(g p) c -> p g c", g=4))

            if i < last:
                nc.sync.dma_start(ag_in[i][:], yT_all[:])
                nc.gpsimd.collective_compute(
                    "AllGather", ALU.bypass, replica_groups=groups,
                    ins=[ag_in[i][:]], outs=[ag_out[i][:]])
            else:
                # second half on the ACT queue (its exp work is already done)
                nc.scalar.dma_start(ag_in3[1][:], yT_all[:, 2 * tt:4 * tt])
                nc.gpsimd.collective_compute(
                    "AllGather", ALU.bypass, replica_groups=groups,
                    ins=[ag_in3[1][:]], outs=[ag_out3[1][:]])
                yf3[1] = rg.tile([128, 2 * HLOC * tt], BF16, name="y3b",
                                 tag="y3b", bufs=1)
                nc.sync.dma_start(
                    yf3[1][:],
                    ag_out3[1][:].rearrange("(g p) c -> p g c", g=4))

            # ================= outproj(i-1) =================
            if i >= 1:
                k = i - 1
                yfk = yf[k]
                ot = rg.tile([128, 4 * DH], F32, name="ot", tag="ot", bufs=2)
                for t in range(4):
                    psx = ps.tile([128, tt], F32, name="psx", tag="stx",
                                  bufs=2)
                    for c in range(NKC):
                        g, hh = c // 4, c % 4
                        col = 4 * tt * g + tt * hh + 128 * t
                        nc.tensor.matmul(
                            psx[:, 0:DH], yfk[:, col:col + 128],
                            wo_all[:, DH * c:DH * (c + 1)],
                            start=(c == 0), stop=(c == NKC - 1))
                    nc.vector.tensor_copy(ot[:, DH * t:DH * (t + 1)],
                                          psx[:, 0:DH])
                nc.sync.dma_start(out[128 * k:128 * (k + 1), :], ot[:])

        # ----- outproj(last): c-outer with 4 live accumulators; the chunks
        # covered by the first-half AllGather run first -----
        accs = [ps.tile([128, tt], F32, name=f"fo{t}",
                        tag=("stx" if t < 2 else f"o2{t - 2}"),
                        bufs=(2 if t < 2 else 1)) for t in range(4)]
        ot = rg.tile([128, 4 * DH], F32, name="ot", tag="ot", bufs=2)
        order = ([c for c in range(NKC) if c % 4 < 2]
                 + [c for c in range(NKC) if c % 4 >= 2])
        for ci, c in enumerate(order):
            g, hh = c // 4, c % 4
            y3 = yf3[hh // 2]               # half x holds heads 2x, 2x+1
            hcol = 2 * tt * g + tt * (hh % 2)
            for t in range(4):
                nc.tensor.matmul(
                    accs[t][:, 0:DH],
                    y3[:, hcol + 128 * t:hcol + 128 * (t + 1)],
                    wo_all[:, DH * c:DH * (c + 1)],
                    start=(ci == 0), stop=(ci == NKC - 1))
        for t in range(4):
            nc.vector.tensor_copy(ot[:, DH * t:DH * (t + 1)],
                                  accs[t][:, 0:DH])
        k = nt - 1
        nc.sync.dma_start(out[128 * k:128 * (k + 1), 0:2 * DH],
                          ot[:, 0:2 * DH])
        nc.scalar.dma_start(out[128 * k:128 * (k + 1), 2 * DH:4 * DH],
                            ot[:, 2 * DH:4 * DH])


_NC_CACHE = {}


def get_nc(t_seq=T, n_reps=1):
    key = (t_seq, n_reps)
    if key not in _NC_CACHE:
        _NC_CACHE[key] = build_nc(t_seq, n_reps)
    return _NC_CACHE[key]


def make_in_maps(x, cos, sin, Wq, Wk, Wv, Wo, t_seq=T):
    import ml_dtypes
    BF = ml_dtypes.bfloat16
    nt = t_seq // TT
    half = D // 2
    cosT = np.asarray(cos, np.float32).reshape(t_seq, half).T
    sinT = np.asarray(sin, np.float32).reshape(t_seq, half).T
    cosPk = np.ascontiguousarray(np.concatenate([cosT, cosT], 0)).astype(BF)
    sinPk = np.ascontiguousarray(np.concatenate([sinT, -sinT], 0)).astype(BF)

    def packw(Wg, ow):
        Wg = np.asarray(Wg, np.float32)
        return np.ascontiguousarray(
            Wg.T.reshape(NKC, 128, ow).transpose(1, 0, 2).reshape(128, NKC * ow)
        ).astype(BF)

    wqPk = [packw(Wq[DH * g:DH * (g + 1)], DH) for g in range(4)]
    wkPk = [packw(Wk[D * g:D * (g + 1)], D) for g in range(4)]
    wvPk = [packw(Wv[D * g:D * (g + 1)], D) for g in range(4)]
    woPk = [packw(Wo[DH * g:DH * (g + 1)], DH) for g in range(4)]
    xPk = []
    for b in range(x.shape[0]):
        xb = np.asarray(x[b], np.float32).T        # [C, t_seq]
        xp = xb.reshape(NKC, 128, nt, TT).transpose(1, 2, 0, 3)
        xPk.append(np.ascontiguousarray(
            xp.reshape(128, nt * NKC * TT)).astype(BF))

    in_maps = []
    for c in range(N_CORES):
        b, g = c // 4, c % 4
        in_maps.append({
            "xP": xPk[b], "wqP": wqPk[g], "wkP": wkPk[g], "wvP": wvPk[g],
            "woP": woPk[g], "cosP": cosPk, "sinP": sinPk,
        })
    return in_maps


def kernel(x, cos, sin, Wq, Wk, Wv, Wo):
    x = np.asarray(x, dtype=np.float32)
    nc = get_nc(T)
    in_maps = make_in_maps(x, np.asarray(cos), np.asarray(sin),
                           np.asarray(Wq), np.asarray(Wk), np.asarray(Wv),
                           np.asarray(Wo), T)
    res = run_bass_kernel_spmd(nc, in_maps, core_ids=list(range(N_CORES)))
    outa = np.empty((B, T, C), dtype=np.float32)
    for c in range(N_CORES):
        b, g = c // 4, c % 4
        o = np.asarray(res.results[c]["out"], np.float32)
        o = o.reshape(NT, 128, 4, DH).transpose(0, 2, 1, 3).reshape(T, DH)
        outa[b, :, DH * g:DH * (g + 1)] = o
    return outa
